# revision 10
# baseline (speedup 1.0000x reference)
"""Distributed GCNII-style graph convolution on 8 Trainium2 NeuronCores.

reference:
    msgs    = features[edge_src] * edge_vals[:, None]
    hi      = segment_sum(msgs, edge_dst, N)
    support = (1-ALPHA)*hi + ALPHA*features0
    out     = relu(BETA*(support @ W) + (1-BETA)*support)
            = relu(support @ W'),  W' = BETA*W + (1-BETA)*I

sharding: nodes (rows) split across 8 cores by edge_dst.  Within a core,
nodes are greedily bin-packed into tiles of <=TILE nodes such that each
(tile, src%4 residue) class holds <=128 edges -- so every class is exactly
one 128-edge chunk and descriptor padding stays ~12% (vs +64% for fixed
64-node tiles).  `features` is replicated to every core so the src gather is
local (the "all-gather" happens at input-distribution time).

gather: the HW `dma_gather` instruction takes int16 indices, so the
[100000, 64] f32 table is addressed as 25000 4-row units (stride 1 KB).
Edges in residue class r = src%4 gather 64 f32 at unit src//4 with base
offset r*64 elements.  One dma_gather call per (group-of-8-tiles, residue),
spread across the 4 SWDGE queues (queue_num=r); indices are wrapped 16-wide
and replicated to 128 partitions as the ucode expects.  Each call is 1024
indices = exactly 64 descriptors per SDMA engine, emitted with
single_packet=True so each engine's stream coalesces into one packet
(first/concatenate/last) instead of 64 singleton packets -- fewer DMA
events and less per-packet overhead on the m2s/s2m bus.

per-core device program (SPMD, one Bass program):
  - gather G[p, c, :] = features[src[p, c], :]      (POOL dma_gather, 4 queues)
  - A[p, c, n] = 0.9*val[p,c] * (dstcol[p,c] == n)  (DVE iota-compare, x val;
    keep BOTH ops on DVE -- running the mult on GPSIMD looks good in CoreSim
    but is 4x slower on HW: Q7 tensor ops contend with SWDGE descriptor gen)
  - PSUM[64f, 384n] += G_chunk.T @ A_chunk          (PE, per 384-node group)
  - support_T = PSUM + 0.1*features0_T_slice        (DVE)
  - out_T = relu(W'.T @ support_T)                  (PE + ACT)
  - transposed [feature, node] layout throughout; host untransposes and
    un-permutes the packed node order.
"""

import os
import sys

import numpy as np


def _import_concourse():
    try:
        import concourse  # noqa: F401
    except ImportError:
        for p in ("/opt/trn_rl_repo", "/root/.axon_site/_ro/trn_rl_repo"):
            if os.path.isdir(p) and p not in sys.path:
                sys.path.insert(0, p)
        import concourse  # noqa: F401


# problem constants (hardcoded; harness gives full-size inputs)
N_NODES = 100000
N_EDGES = 1000000
F = 64
ALPHA = 0.1
BETA = 0.5
N_CORES = 8

TILE = 48          # max nodes per tile (A matrix width)
GROUP_TILES = 8    # tiles per PSUM group -> 384 nodes per group (<=512 f32);
                   # also 8*128 idxs/gather = 64 descs/engine = one full packet
P = 128            # SBUF partitions / edges per chunk
R = 4              # src residue classes (int16 index limit workaround)


def _pack_tiles(deg):
    """Greedy sequential packing: nodes -> tiles with <=TILE nodes and
    <=P edges per residue class.  deg: [shard, R] int.  Returns
    (tile_of_node, pos_of_node, ntiles)."""
    shard = deg.shape[0]
    tile_of = np.empty(shard, np.int32)
    pos_of = np.empty(shard, np.int32)
    cnt = np.zeros(R, np.int64)
    t, nn = 0, 0
    for i in range(shard):
        d = deg[i]
        if nn + 1 > TILE or np.any(cnt + d > P):
            t += 1
            cnt = d.astype(np.int64).copy()
            nn = 1
            tile_of[i], pos_of[i] = t, 0
        else:
            tile_of[i], pos_of[i] = t, nn
            cnt += d
            nn += 1
    return tile_of, pos_of, t + 1


def _prep(features, features0, edge_src, edge_dst, edge_vals, W,
          n_nodes=N_NODES, n_cores=N_CORES):
    """Host-side sharding.  Returns (in_maps, T, node_cols)."""
    f32 = np.float32
    assert n_nodes % R == 0
    shard = n_nodes // n_cores

    core = np.clip(edge_dst // shard, 0, n_cores - 1)
    dst_local = edge_dst - core * shard
    res = edge_src % R

    # per-core greedy tile packing
    tile_of = np.empty(n_nodes, np.int32)
    pos_of = np.empty(n_nodes, np.int32)
    ntiles = []
    for c in range(n_cores):
        deg = np.zeros((shard, R), np.int32)
        m = core == c
        np.add.at(deg, (dst_local[m], res[m]), 1)
        tl, ps, nt = _pack_tiles(deg)
        sl = slice(c * shard, (c + 1) * shard)
        tile_of[sl], pos_of[sl] = tl, ps
        ntiles.append(nt)
    T = ((max(ntiles) + GROUP_TILES - 1) // GROUP_TILES) * GROUP_TILES
    NCHUNK = GROUP_TILES * R
    NCOL = T * R                      # total chunks per core

    etile = tile_of[edge_dst]         # tile of edge's dst (within its core)
    # chunk column: g*NCHUNK + r*GROUP_TILES + t_local
    col = ((etile // GROUP_TILES) * NCHUNK + res * GROUP_TILES
           + etile % GROUP_TILES)
    key = core * NCOL + col
    counts = np.bincount(key, minlength=n_cores * NCOL)
    assert counts.max() <= P, "tile packing violated chunk capacity"
    order = np.argsort(key, kind="stable")
    sk = key[order]
    starts = np.concatenate([[0], np.cumsum(counts)[:-1]])
    part = np.arange(len(sk), dtype=np.int64) - starts[sk]
    col_s = sk % NCOL
    core_s = sk // NCOL

    unit_all = np.zeros((n_cores, P, NCOL), np.int32)
    dst_all = np.zeros((n_cores, P, NCOL), f32)
    val_all = np.zeros((n_cores, P, NCOL), f32)
    unit_all[core_s, part, col_s] = edge_src[order] // R
    dst_all[core_s, part, col_s] = pos_of[edge_dst[order]].astype(f32)
    val_all[core_s, part, col_s] = ((1.0 - ALPHA) * edge_vals[order]
                                    ).astype(f32)

    # idx16: per (group, residue) call covering chunk cols
    # [g*NCHUNK + r*GROUP_TILES, +GROUP_TILES); flat list i = chunk*128+p;
    # ucode reads list element i from partition i%16, column i//16,
    # replicated across the 8 16-row blocks.
    idx16_all = np.zeros((n_cores, P, NCOL * P // 16), np.int16)
    for cidx in range(n_cores):
        blocks = []
        grid = unit_all[cidx]
        for g in range(T // GROUP_TILES):
            for r in range(R):
                c0 = g * NCHUNK + r * GROUP_TILES
                flat = grid[:, c0:c0 + GROUP_TILES].T.ravel()
                blk = flat.reshape(-1, 16).T
                blocks.append(np.tile(blk, (8, 1)))
        idx16_all[cidx] = np.concatenate(blocks, axis=1).astype(np.int16)

    Wp = (BETA * W + (1.0 - BETA) * np.eye(F, dtype=f32)).astype(f32)
    iota = np.broadcast_to(np.arange(TILE, dtype=f32), (P, TILE)).copy()
    feat = np.ascontiguousarray(features, dtype=f32)

    in_maps = []
    node_cols = []                    # per core: output column of each node
    for c in range(n_cores):
        sl = slice(c * shard, (c + 1) * shard)
        cols = tile_of[sl].astype(np.int64) * TILE + pos_of[sl]
        node_cols.append(cols)
        f0sT = np.zeros((F, T * TILE), f32)
        f0sT[:, cols] = (ALPHA * features0[sl]).T
        in_maps.append({
            "features": feat,
            "eidx": np.ascontiguousarray(idx16_all[c]),
            "edst": np.ascontiguousarray(dst_all[c]),
            "eval": np.ascontiguousarray(val_all[c]),
            "f0sT": f0sT,
            "Wp": Wp,
            "iota": iota,
        })
    return in_maps, T, node_cols


def _build(T, n_nodes=N_NODES, passes=1, skip=(),
           gather_elem=F, gather_queues=R, gather_res=R):
    """Build the SPMD Bass/Tile program.  Returns nc (unfinalized)."""
    from contextlib import ExitStack

    from concourse import bacc, mybir, tile
    from concourse.bass import AP

    f32, i16 = mybir.dt.float32, mybir.dt.int16
    NCOL = T * R
    NG = T // GROUP_TILES                        # groups per core
    NCHUNK = GROUP_TILES * R                     # chunks per group
    GN = TILE * GROUP_TILES                      # nodes per group (480)
    WIDTH = T * TILE                             # outT columns
    IDX16 = NCOL * P // 16
    n_units = n_nodes // R

    nc = bacc.Bacc(num_swdge_queues=4)
    feat_d = nc.dram_tensor("features", [n_nodes, F], f32, kind="ExternalInput")
    idx_d = nc.dram_tensor("eidx", [P, IDX16], i16, kind="ExternalInput")
    dst_d = nc.dram_tensor("edst", [P, NCOL], f32, kind="ExternalInput")
    val_d = nc.dram_tensor("eval", [P, NCOL], f32, kind="ExternalInput")
    f0_d = nc.dram_tensor("f0sT", [F, WIDTH], f32, kind="ExternalInput")
    w_d = nc.dram_tensor("Wp", [F, F], f32, kind="ExternalInput")
    iota_d = nc.dram_tensor("iota", [P, TILE], f32, kind="ExternalInput")
    out_d = nc.dram_tensor("outT", [F, WIDTH], f32, kind="ExternalOutput")
    feat_ap = feat_d[:]

    with tile.TileContext(nc) as tc, ExitStack() as ctx:
        const = ctx.enter_context(tc.tile_pool(name="const", bufs=1))
        gpool = ctx.enter_context(tc.tile_pool(name="g", bufs=3))
        apool = ctx.enter_context(tc.tile_pool(name="a", bufs=2))
        spool = ctx.enter_context(tc.tile_pool(name="sup", bufs=2))
        opool = ctx.enter_context(tc.tile_pool(name="o", bufs=2))
        pspool = ctx.enter_context(tc.tile_pool(name="ps", bufs=2, space="PSUM"))
        ps2pool = ctx.enter_context(tc.tile_pool(name="ps2", bufs=2,
                                                 space="PSUM"))

        idx_sb = const.tile([P, IDX16], i16)
        dst_sb = const.tile([P, NCOL], f32)
        val_sb = const.tile([P, NCOL], f32)
        f0_sb = const.tile([F, WIDTH], f32)
        w_sb = const.tile([F, F], f32)
        iota_sb = const.tile([P, TILE], f32)
        nc.sync.dma_start(idx_sb[:], idx_d[:])
        nc.sync.dma_start(dst_sb[:], dst_d[:])
        nc.sync.dma_start(val_sb[:], val_d[:])
        nc.sync.dma_start(f0_sb[:], f0_d[:])
        nc.sync.dma_start(w_sb[:], w_d[:])
        nc.sync.dma_start(iota_sb[:], iota_d[:])

        iota_ap = iota_sb[:]
        for _pass in range(passes):
          for g in range(NG):
              col0 = g * NCHUNK
              gt = gpool.tile([P, NCHUNK, F], f32)
              if 'gather' in skip:
                  nc.sync.dma_start(gt[:, 0, :], f0_d[:P, :F])
              else:
                  for r in range(gather_res):
                      num_idxs = GROUP_TILES * P
                      off16 = (g * NCHUNK + r * GROUP_TILES) * P // 16
                      src_ap = AP(feat_ap.tensor, r * F,
                                  [[R * F, n_units], [1, F]])
                      nc.gpsimd.dma_gather(
                          out_ap=gt[:, r * GROUP_TILES:(r + 1) * GROUP_TILES,
                                    :gather_elem],
                          in_ap=src_ap,
                          idxs_ap=idx_sb[:, off16:off16 + num_idxs // 16],
                          num_idxs=num_idxs,
                          num_idxs_reg=num_idxs,
                          elem_size=gather_elem,
                          elem_step=R * F,
                          single_packet=True,
                          queue_num=r % gather_queues,
                      )

              at = apool.tile([P, NCHUNK, TILE], f32)
              iota_bc = AP(iota_ap.tensor, iota_ap.offset,
                           [iota_ap.ap[0], [0, NCHUNK], iota_ap.ap[1]])
              dst_bc = dst_sb[:, col0:col0 + NCHUNK].broadcast_to(
                  [P, NCHUNK, TILE])
              val_bc = val_sb[:, col0:col0 + NCHUNK].broadcast_to(
                  [P, NCHUNK, TILE])
              if 'abuild' not in skip:
                  nc.vector.tensor_tensor(out=at[:], in0=iota_bc, in1=dst_bc,
                                          op=mybir.AluOpType.is_equal)
                  nc.vector.tensor_tensor(out=at[:], in0=at[:], in1=val_bc,
                                          op=mybir.AluOpType.mult)

              psg = pspool.tile([F, GN], f32)
              if 'mm' in skip:
                  nc.vector.tensor_copy(psg[:, :TILE], at[:F, 0, :])
              else:
                  # tile-major emission: each tile's accumulation group
                  # (start at r=0, stop at r=R-1) closes before the next opens
                  for j in range(GROUP_TILES):
                      for r in range(R):
                          q = r * GROUP_TILES + j
                          nc.tensor.matmul(
                              out=psg[:, j * TILE:(j + 1) * TILE],
                              lhsT=gt[:, q, :],
                              rhs=at[:, q, :],
                              start=(r == 0),
                              stop=(r == R - 1),
                          )

              sup = spool.tile([F, GN], f32)
              nc.vector.tensor_add(sup[:], psg[:],
                                   f0_sb[:, g * GN:(g + 1) * GN])

              ps2 = ps2pool.tile([F, GN], f32)
              nc.tensor.matmul(ps2[:], lhsT=w_sb[:], rhs=sup[:],
                               start=True, stop=True)

              ot = opool.tile([F, GN], f32)
              nc.scalar.activation(ot[:], ps2[:],
                                   mybir.ActivationFunctionType.Relu)
              nc.sync.dma_start(out_d[:, g * GN:(g + 1) * GN], ot[:])

    return nc


def kernel(features, features0, edge_src, edge_dst, edge_vals, W):
    _import_concourse()
    from concourse.bass_utils import run_bass_kernel_spmd

    features = np.asarray(features, np.float32)
    features0 = np.asarray(features0, np.float32)
    edge_src = np.asarray(edge_src, np.int32)
    edge_dst = np.asarray(edge_dst, np.int32)
    edge_vals = np.asarray(edge_vals, np.float32)
    W = np.asarray(W, np.float32)

    in_maps, T, node_cols = _prep(
        features, features0, edge_src, edge_dst, edge_vals, W)
    nc = _build(T)
    nc.finalize()
    res = run_bass_kernel_spmd(nc, in_maps, list(range(N_CORES)))
    outs = []
    for i in range(N_CORES):
        outT = res.results[i]["outT"]            # [F, T*TILE]
        outs.append(outT[:, node_cols[i]].T)
    return np.ascontiguousarray(np.concatenate(outs, axis=0), dtype=np.float32)

